# revision 1
# baseline (speedup 1.0000x reference)
"""Compact Bilinear Pooling on 8 Trainium2 NeuronCores (Bass/Tile).

Math: cbp = irfft(sum_l rfft(sketch1_l) * rfft(sketch2_l)), sum over the 196
spatial locations folded inside the spectral product (linearity of ifft), then
signed-sqrt + L2 norm.  The count-sketch + rFFT pair is algebraically a dense
matmul F_j = A_j @ x_flat with A_j[k,c] = s_j[c] * exp(-2i*pi*k*h_j[c]/8192)
(the A_j depend only on the tiny hash vectors and are built host-side).  The
inverse rFFT of the single 8192-vector per batch is done on-device with a
split-radix (64x64 outer/inner) pair of small matmuls plus twiddles.

Sharding: pure batch data-parallel, batch 16 -> 2 per core, no collectives.
Precision: bf16 weights/inputs, fp32 PSUM accumulation (rel err ~5e-3).
"""

import numpy as np
import ml_dtypes

import concourse.bass as bass
import concourse.mybir as mybir
import concourse.tile as tile
from concourse import bacc
from concourse.bass_utils import run_bass_kernel_spmd

F32 = mybir.dt.float32
BF16 = mybir.dt.bfloat16
AX = mybir.AluOpType

B, C, H, W = 16, 2048, 14, 14
L = H * W                  # 196
D = 8192
NCORES = 8
BPC = B // NCORES          # 2 batches per core
NL = BPC * L               # 392 moving columns per core
KT = C // 128              # 16 contraction subtiles
NCHUNK = 33                # ceil(4097/128) rfft row chunks
KPAD = NCHUNK * 128        # 4224

# consts tensor column offsets (all f32, 128 partitions)
O_E1R, O_E1I, O_E1IN = 0, 128, 256          # stage1 rhs (128,128) each
O_TWR, O_TWI = 384, 512                     # twiddles (rows 0:64 valid)
O_E2RS, O_E2IN = 640, 704                   # stage3 rhs (128,64) each
O_SGNR = 768                                # row 0: ((-1)^q)/D over q (1,128)
O_OMAT = 896                                # ones matrix (128,128)
O_EPS = 1024                                # 1e-5 col (128,1)
O_ZERO = 1025                               # zeros col (128,1)
NCONST = 1026


def _build_program():
    nc = bacc.Bacc()
    xin = nc.dram_tensor("xin", [128, KT, NL], BF16, kind="ExternalInput")
    wts = nc.dram_tensor("wts", [NCHUNK, 4, 128, KT, 128], BF16,
                         kind="ExternalInput")
    cst = nc.dram_tensor("cst", [128, NCONST], F32, kind="ExternalInput")
    outd = nc.dram_tensor("out", [BPC, D], F32, kind="ExternalOutput")
    gr_d = nc.dram_tensor("gr_scratch", [BPC, KPAD], F32)
    gi_d = nc.dram_tensor("gi_scratch", [BPC, KPAD], F32)

    with tile.TileContext(nc) as tc:
        with (
            tc.tile_pool(name="xp", bufs=1) as xp,
            tc.tile_pool(name="wp", bufs=3) as wp,
            tc.tile_pool(name="gp", bufs=1) as gp,
            tc.tile_pool(name="tp", bufs=2) as tp,
        ):
            xt = xp.tile([128, KT, NL], BF16, name="xt")
            nc.sync.dma_start(xt[:], xin[:])
            cs = xp.tile([128, NCONST], F32, name="cs")
            nc.sync.dma_start(cs[:], cst[:])

            grt = gp.tile([128, NCHUNK, BPC], F32, name="grt")
            git = gp.tile([128, NCHUNK, BPC], F32, name="git")
            dummy = gp.tile([128, L], F32, name="dummy")

            # ---------------- forward: F chunks + spectral product ----------
            fwd_ps = tc.tile_pool(name="psF", bufs=2, space="PSUM")
            psp = fwd_ps.__enter__()
            for j in range(NCHUNK):
                wt = [wp.tile([128, KT, 128], BF16, name=f"wt{w}",
                              tag=f"wt{w}") for w in range(4)]
                for w in range(4):
                    nc.sync.dma_start(wt[w][:], wts[j, w])
                ps = [psp.tile([128, NL], F32, name=f"mm{w}", tag=f"mm{w}")
                      for w in range(4)]
                for w in range(4):
                    for kt in range(KT):
                        nc.tensor.matmul(ps[w][:], lhsT=wt[w][:, kt, :],
                                         rhs=xt[:, kt, :],
                                         start=(kt == 0), stop=(kt == KT - 1))
                # G[k] = sum_l (P1+iQ1)(P2+iQ2) per batch.  DVE may read only
                # one PSUM operand per op -> stage P2/Q2 into SBUF via ACT.
                cp2 = tp.tile([128, NL], F32, name="cp2", tag="cp2")
                cq2 = tp.tile([128, NL], F32, name="cq2", tag="cq2")
                nc.vector.tensor_copy(cp2[:], ps[2][:])
                nc.vector.tensor_copy(cq2[:], ps[3][:])
                sb = {0: None, 1: None, 2: cp2, 3: cq2}
                acc = tp.tile([128, 8], F32, name="acc", tag="acc")
                for b in range(BPC):
                    sl = slice(b * L, (b + 1) * L)
                    pairs = ((0, 2), (1, 3), (0, 3), (1, 2))  # P1P2,Q1Q2,P1Q2,Q1P2
                    for i, (a_, b_) in enumerate(pairs):
                        nc.vector.scalar_tensor_tensor(
                            dummy[:], ps[a_][:, sl], 1.0, sb[b_][:, sl],
                            op0=AX.mult, op1=AX.mult,
                            accum_out=acc[:, 4 * b + i: 4 * b + i + 1])
                for b in range(BPC):
                    o = 4 * b
                    nc.vector.tensor_sub(grt[:, j, b:b + 1],
                                         acc[:, o:o + 1], acc[:, o + 1:o + 2])
                    nc.vector.tensor_add(git[:, j, b:b + 1],
                                         acc[:, o + 2:o + 3], acc[:, o + 3:o + 4])

            for b in range(BPC):
                nc.sync.dma_start(gr_d[b].rearrange("(c p) -> p c", p=128),
                                  grt[:, :, b])
                nc.sync.dma_start(gi_d[b].rearrange("(c p) -> p c", p=128),
                                  git[:, :, b])
            fwd_ps.__exit__(None, None, None)

            # ---------------- inverse rFFT + epilogue, per batch ------------
            inv_ps = tc.tile_pool(name="psI", bufs=2, space="PSUM")
            psp = inv_ps.__enter__()
            for b in range(BPC):
                gtr = tp.tile([128, 64], F32, name="gtr", tag="gtr")
                gti = tp.tile([128, 64], F32, name="gti", tag="gti")
                nc.vector.memset(gtr[:], 0.0)
                nc.vector.memset(gti[:], 0.0)
                nc.sync.dma_start(
                    gtr[0:64, :], gr_d[b, 0:4096].rearrange("(u v) -> u v", v=64))
                nc.sync.dma_start(
                    gti[0:64, :], gi_d[b, 0:4096].rearrange("(u v) -> u v", v=64))

                # stage 1: T[v,q] = sum_u G[64u+v] * e1[u,q]
                ptr = psp.tile([64, 128], F32, name="ptr", tag="ptr")
                pti = psp.tile([64, 128], F32, name="pti", tag="pti")
                nc.tensor.matmul(ptr[:], lhsT=gtr[:, :], rhs=cs[:, O_E1R:O_E1R + 128],
                                 start=True, stop=False)
                nc.tensor.matmul(ptr[:], lhsT=gti[:, :], rhs=cs[:, O_E1IN:O_E1IN + 128],
                                 start=False, stop=True)
                nc.tensor.matmul(pti[:], lhsT=gtr[:, :], rhs=cs[:, O_E1I:O_E1I + 128],
                                 start=True, stop=False)
                nc.tensor.matmul(pti[:], lhsT=gti[:, :], rhs=cs[:, O_E1R:O_E1R + 128],
                                 start=False, stop=True)

                # twiddle: T' = T * exp(2i pi v q / 8192)
                tpr = tp.tile([128, 128], F32, name="tpr", tag="tpr")
                tpi = tp.tile([128, 128], F32, name="tpi", tag="tpi")
                tmp = tp.tile([64, 128], F32, name="tmp", tag="tmp")
                nc.vector.memset(tpr[:], 0.0)
                nc.vector.memset(tpi[:], 0.0)
                nc.vector.tensor_mul(tpr[0:64, :], ptr[:], cs[0:64, O_TWR:O_TWR + 128])
                nc.vector.tensor_mul(tmp[:], pti[:], cs[0:64, O_TWI:O_TWI + 128])
                nc.vector.tensor_sub(tpr[0:64, :], tpr[0:64, :], tmp[:])
                nc.vector.tensor_mul(tpi[0:64, :], ptr[:], cs[0:64, O_TWI:O_TWI + 128])
                nc.vector.tensor_mul(tmp[:], pti[:], cs[0:64, O_TWR:O_TWR + 128])
                nc.vector.tensor_add(tpi[0:64, :], tpi[0:64, :], tmp[:])

                # corr row: tpr[64, q] = (-Gr[0] + (-1)^q Gr[4096])/D paired with
                # ones in e2rs row 64 -> folded into the stage-3 matmul.
                g2 = tp.tile([1, 2], F32, name="g2", tag="g2")
                nc.sync.dma_start(g2[0:1, :], gr_d[b, 0:4097:4096])
                crow = tp.tile([1, 128], F32, name="crow", tag="crow")
                nc.vector.tensor_mul(crow[:], cs[0:1, O_SGNR:O_SGNR + 128],
                                     g2[0:1, 1:2].to_broadcast([1, 128]))
                nc.vector.scalar_tensor_tensor(
                    crow[:], g2[0:1, 0:1].to_broadcast([1, 128]), -1.0 / D,
                    crow[:], op0=AX.mult, op1=AX.add)
                nc.sync.dma_start(tpr[64:65, :], crow[:])

                # stage 3: S[q,p] = sum_v T'[v,q] e2[v,p]   (scaled by 2/D)
                pss = psp.tile([128, 64], F32, name="pss", tag="pss")
                nc.tensor.matmul(pss[:], lhsT=tpr[:], rhs=cs[:, O_E2RS:O_E2RS + 64],
                                 start=True, stop=False)
                nc.tensor.matmul(pss[:], lhsT=tpi[:], rhs=cs[:, O_E2IN:O_E2IN + 64],
                                 start=False, stop=True)

                # signed sqrt: sign(x) * sqrt(|x| + 1e-5)
                sq = tp.tile([128, 64], F32, name="sq", tag="sq")
                sg = tp.tile([128, 64], F32, name="sg", tag="sg")
                ss = tp.tile([128, 64], F32, name="ss", tag="ss")
                nc.scalar.activation(sq[:], pss[:], mybir.ActivationFunctionType.Abs,
                                     bias=cs[:, O_ZERO:O_ZERO + 1])
                nc.scalar.activation(sq[:], sq[:], mybir.ActivationFunctionType.Sqrt,
                                     bias=cs[:, O_EPS:O_EPS + 1])
                nc.scalar.sign(sg[:], pss[:], bias=cs[:, O_ZERO:O_ZERO + 1])
                nc.vector.tensor_mul(ss[:], sq[:], sg[:])

                # L2 normalize: scale = 1/max(sqrt(sum ss^2), 1e-12)
                s2 = tp.tile([128, 64], F32, name="s2", tag="s2")
                rs = tp.tile([128, 1], F32, name="rs", tag="rs")
                nc.vector.tensor_mul(s2[:], ss[:], ss[:])
                nc.vector.reduce_sum(rs[:], s2[:], axis=mybir.AxisListType.X)
                pn = psp.tile([128, 1], F32, name="pn", tag="pn")
                nc.tensor.matmul(pn[:], lhsT=cs[:, O_OMAT:O_OMAT + 128], rhs=rs[:],
                                 start=True, stop=True)
                nrm = tp.tile([128, 1], F32, name="nrm", tag="nrm")
                inv = tp.tile([128, 1], F32, name="inv", tag="inv")
                nc.scalar.activation(nrm[:], pn[:], mybir.ActivationFunctionType.Sqrt,
                                     bias=cs[:, O_ZERO:O_ZERO + 1])
                nc.vector.tensor_scalar_max(nrm[:], nrm[:], 1e-12)
                nc.vector.reciprocal(inv[:], nrm[:])
                res = tp.tile([128, 64], F32, name="res", tag="res")
                nc.scalar.mul(res[:], ss[:], inv[:])

                nc.sync.dma_start(outd[b].rearrange("(p q) -> q p", q=128), res[:])
            inv_ps.__exit__(None, None, None)
    nc.finalize()
    return nc


_PROGRAM = None


def _get_program():
    global _PROGRAM
    if _PROGRAM is None:
        _PROGRAM = _build_program()
    return _PROGRAM


def _build_weights(h1, h2, s1, s2):
    """wts (NCHUNK, 4, 128p, KT, 128m) bf16 with A[k,c]=s[c]*exp(-2i pi k h[c]/D)."""
    k = np.arange(KPAD, dtype=np.int64)[:, None]
    tab = np.arange(D, dtype=np.float64) * (2.0 * np.pi / D)
    cos_t = np.cos(tab).astype(np.float32)
    sin_t = np.sin(tab).astype(np.float32)
    mats = []
    for h, s in ((h1, s1), (h2, s2)):
        idx = (k * h[None, :].astype(np.int64)) % D
        sg = (2 * s - 1).astype(np.float32)[None, :]
        Ar = cos_t[idx] * sg
        Ai = -sin_t[idx] * sg
        Ar[4097:] = 0.0
        Ai[4097:] = 0.0
        mats += [Ar, Ai]
    # (KPAD, C) -> (NCHUNK, 128m, KT, 128p) -> (NCHUNK, 128p, KT, 128m)
    out = np.empty((NCHUNK, 4, 128, KT, 128), dtype=ml_dtypes.bfloat16)
    for w, A in enumerate(mats):
        t = A.reshape(NCHUNK, 128, KT, 128).transpose(0, 3, 2, 1)
        out[:, w] = t.astype(ml_dtypes.bfloat16)
    return out


def _build_consts():
    cst = np.zeros((128, NCONST), np.float32)
    q = np.arange(128)
    u = np.arange(64)
    v = np.arange(64)
    p = np.arange(64)
    cst[0:64, O_E1R:O_E1R + 128] = np.cos(2 * np.pi * np.outer(u, q) / 128)
    e1i = np.sin(2 * np.pi * np.outer(u, q) / 128)
    cst[0:64, O_E1I:O_E1I + 128] = e1i
    cst[0:64, O_E1IN:O_E1IN + 128] = -e1i
    cst[0:64, O_TWR:O_TWR + 128] = np.cos(2 * np.pi * np.outer(v, q) / D)
    cst[0:64, O_TWI:O_TWI + 128] = np.sin(2 * np.pi * np.outer(v, q) / D)
    cst[0:64, O_E2RS:O_E2RS + 64] = np.cos(2 * np.pi * np.outer(v, p) / 64) * (2.0 / D)
    cst[0:64, O_E2IN:O_E2IN + 64] = -np.sin(2 * np.pi * np.outer(v, p) / 64) * (2.0 / D)
    cst[64, O_E2RS:O_E2RS + 64] = 1.0          # pairs with corr row in tpr[64]
    cst[0, O_SGNR:O_SGNR + 128] = ((-1.0) ** q) / D
    cst[:, O_OMAT:O_OMAT + 128] = 1.0
    cst[:, O_EPS] = 1e-5
    cst[:, O_ZERO] = 0.0
    return cst


def _build_in_maps(x, h1, h2, s1, s2):
    x = np.asarray(x, np.float32)
    h1 = np.asarray(h1, np.int32)
    h2 = np.asarray(h2, np.int32)
    s1 = np.asarray(s1, np.int32)
    s2 = np.asarray(s2, np.int32)
    wts = _build_weights(h1, h2, s1, s2)
    cst = _build_consts()
    in_maps = []
    xr = x.reshape(B, C, L)
    for i in range(NCORES):
        x2 = np.concatenate([xr[BPC * i + b] for b in range(BPC)], axis=1)  # (C, NL)
        xdev = np.ascontiguousarray(
            x2.reshape(KT, 128, NL).transpose(1, 0, 2)).astype(ml_dtypes.bfloat16)
        in_maps.append({"xin": xdev, "wts": wts, "cst": cst})
    return in_maps


def _run(x, h1, h2, s1, s2, trace=False, tmpdir=None):
    nc = _get_program()
    in_maps = _build_in_maps(x, h1, h2, s1, s2)
    res = run_bass_kernel_spmd(nc, in_maps, list(range(NCORES)),
                               trace=trace, tmpdir=tmpdir)
    out = np.concatenate([np.asarray(r["out"], np.float32) for r in res.results],
                         axis=0)
    return out, res


def kernel(x, h1, h2, s1, s2):
    out, _ = _run(x, h1, h2, s1, s2, trace=False)
    return out.astype(np.asarray(x).dtype)

